# revision 38
# baseline (speedup 1.0000x reference)
"""TRN2 Bass kernel for nn_Attention (RMSNorm + QKV + softmax attention + out-proj).

Sharding: 8 cores = 2 batches x 4 head-pairs. Core c handles batch c//4 and
heads (2*(c%4), 2*(c%4)+1). Each core computes its partial out-projection
(contracting only its 128 rows of dim_inner); host sums the 4 partials per batch.

Per-core pipeline (cost-model-aware layout):
  A) stream tokens [128,512] blocks: RMSNorm stats on DVE, rstd scale, PE
     transpose -> x^T [512, 4096]; Q^T,K^T = W^T x^T (PE);
     V natural [tok, dv] via stationary-x^T matmuls, stored bf16 with a ones
     column per head (softmax denominator comes out on query partitions).
  B) flash attention per (ib=512 queries, head):
     S^T tiles [128 j, 512 i] on PE -> exp split between ACT (table exp) and
     DVE (Schraudolph int-trick exp: one fused mult+add with int32 convert,
     bits reinterpreted as f32) -> PV in the *transposed* layout:
     stationary = P^T tile [128 j, 128 i], moving = V [128 j, 65] bf16,
     out = o[128 i, 65] accumulated over j in a single PSUM bank group.
     Column 64 = softmax denominator l_i per query (on partitions!).
  C) tail per (ib, head): linv = 1/l (DVE, strided over the 4 it-slices),
     scaled copies o*linv -> O_sb [128 i, 128 d]; per ib: PE transpose
     O_sb -> O^T, out-proj out[i,:] = O^T.T @ Wo (contraction over both
     heads at once), copy to SBUF, DMA partial out.

Engine discipline: single-wait walrus constraint handled by _patch_drain +
fix_waits_nc (unchanged from previous version).
"""
import sys
sys.path.insert(0, "/opt/trn_rl_repo")
import numpy as np

B, N, D = 2, 4096, 512
H, DH = 8, 64
DI = H * DH
NCORES = 8
EPS = 1.1920929e-07  # float32 eps (torch nn.RMSNorm default)

# Schraudolph exp constants, bf16 flavor: exp(x) ~= bitcast_bf16(int16(
# x*EXPA + EXPB)). EXPA = 2^7/ln2; EXPB = 127*2^7 - C with C tuned for
# minimax relative error given the truncating f32->i16 conversion
# (~3.3% max rel err incl. bf16 mantissa quantization).
EXPA = 184.6649652337873
EXPB = float(127 * (1 << 7)) - 5.1

# exp work split: out of every 256 consecutive 2-jt groups, this many go to
# the DVE (Schraudolph); the rest go to ACT (exact table exp).
DVE_EXP_OF256 = 92
# engine assignment knobs (tuned against the cost-model timeline)
QT_ON_ACT = True      # QT copy on ACT (else DVE)
XT_DVE_T = (0, 1, 2, 3)  # which t-slices of the xt copy go to DVE (else ACT)
RSQRT_ON_DVE = False
OUT_COPY_DVE_MOD = 2  # out copies: it % 2 < this -> DVE, else ACT (2=all DVE)

_prog_cache = {}


def _patch_drain(tile_mod, mybir):
    """Split the multi-wait tail drain into a chain of single-wait drains
    (this walrus build rejects >1 sync wait per instruction)."""
    if getattr(tile_mod.TileContext, "_drain_patched", False):
        return

    def _patched(self, tick_clock, wait_clock):
        from concourse.vector_clock import ScopedClock
        nc = self.nc
        drain_inst = nc.sync.drain()
        wait_clock.add_sem_waits(drain_inst.ins, ScopedClock({None: tick_clock.global_clock}))
        si = drain_inst.ins.sync_info
        if si is not None and si.on_wait and len(si.on_wait) > 1:
            waits = list(si.on_wait)
            drain_inst.ins.sync_info = mybir.SyncInfo(
                on_wait=waits[:1], on_update=list(si.on_update or []))
            for w in waits[1:]:
                d2 = nc.sync.drain()
                d2.ins.sync_info = mybir.SyncInfo(on_wait=[w], on_update=[])
        nc.all_engine_barrier()
        assert self.sems is not None
        popped = nc._tile_sem_poison_stack.pop()
        assert popped is self._sem_poison
        nc.clear_and_free_semaphores(list(self.sems.allocated().values()))
        nc.all_engine_barrier()

    tile_mod.TileContext._drain_and_barrier = _patched
    tile_mod.TileContext._drain_patched = True


def build_program():
    import concourse.bass as bass
    import concourse.tile as tile
    from concourse import mybir
    from concourse.masks import make_identity

    _patch_drain(tile, mybir)

    F32 = mybir.dt.float32
    F32R = mybir.dt.float32r
    BF16 = mybir.dt.bfloat16
    I16 = mybir.dt.int16
    I32 = mybir.dt.int32
    AF = mybir.ActivationFunctionType
    ALU = mybir.AluOpType

    NIC = N // 512          # 8 chunks of 512 tokens
    NJT = N // 128          # 32 key tiles of 128
    NIB = N // 512          # 8 query blocks of 512

    nc = bass.Bass(trn_type="TRN2", target_bir_lowering=False)

    tok = nc.dram_tensor("tok", [N, D], F32, kind="ExternalInput")
    wq = nc.dram_tensor("wq", [128, 4, 128], BF16, kind="ExternalInput")
    wk = nc.dram_tensor("wk", [128, 4, 128], BF16, kind="ExternalInput")
    wv = nc.dram_tensor("wv", [128, 4, 128], BF16, kind="ExternalInput")
    wo = nc.dram_tensor("wo", [128, 512], F32R, kind="ExternalInput")
    out_part = nc.dram_tensor("out_part", [N, D], F32, kind="ExternalOutput")

    tok_r = tok.rearrange("(ic t p) d -> ic p t d", t=4, p=128)
    out_r = out_part.rearrange("(ib t p) e -> ib p t e", t=4, p=128)

    with tile.TileContext(nc) as tc:
        with tc.tile_pool(name="consts", bufs=1) as consts, \
             tc.tile_pool(name="big", bufs=1) as big, \
             tc.tile_pool(name="wpool", bufs=1) as wpool:

            # ---- constants ----
            ident_f = consts.tile([128, 128], F32)
            make_identity(nc, ident_f)
            ident = consts.tile([128, 128], F32R)
            nc.vector.tensor_copy(ident, ident_f)
            ident_b = consts.tile([128, 128], BF16)
            nc.vector.tensor_copy(ident_b, ident_f)
            eps_t = consts.tile([128, 1], F32)
            nc.vector.memset(eps_t, EPS)
            # preload the Exp ACT table during the otherwise-idle start window
            warm = consts.tile([1, 1], F32)
            nc.scalar.activation(warm, eps_t[0:1, :], AF.Exp)

            # ---- weights ----
            wq_sb = wpool.tile([128, 4, 128], BF16)
            wk_sb = wpool.tile([128, 4, 128], BF16)
            wv_sb = wpool.tile([128, 4, 128], BF16)
            wo_sb = wpool.tile([128, 512], F32R)
            nc.sync.dma_start(out=wq_sb, in_=wq[:, :, :])
            nc.sync.dma_start(out=wk_sb, in_=wk[:, :, :])
            nc.sync.dma_start(out=wv_sb, in_=wv[:, :, :])
            nc.sync.dma_start(out=wo_sb, in_=wo[:, :])

            # ---- persistent big buffers ----
            QT = big.tile([128, N], F32R)       # [2 heads x 64 qdims, n]
            KT = big.tile([128, N], F32R)
            Vb = big.tile([128, NJT, 2, 65], BF16)  # per j-tile: [v(64)|ones] per head
            nc.vector.memset(Vb[:, :, :, 64:65], 1.0)

            from contextlib import ExitStack
            outer_ctx = ExitStack()
            ptp = outer_ctx.enter_context(tc.tile_pool(name="pt_pool", bufs=5))

            # ---------------- phase A ----------------
            with tc.tile_pool(name="ab_sbuf", bufs=4) as abp, \
                 tc.tile_pool(name="ab_stats", bufs=6) as stp, \
                 tc.tile_pool(name="ab_psum", bufs=3, space="PSUM") as abps, \
                 tc.tile_pool(name="qk_psum", bufs=2, space="PSUM") as qkps, \
                 tc.tile_pool(name="v_psum", bufs=2, space="PSUM") as vps_pool, \
                 tc.tile_pool(name="scr_psum", bufs=1, space="PSUM") as scrps:

                # PE joins: absorb each weight-DMA semaphore with a tiny bf16 matmul
                scr = scrps.tile([2, 2], F32, tag="scr", name="scrj")
                for i, wtile in enumerate((wq_sb, wk_sb, wv_sb, wo_sb)):
                    if len(wtile.shape) == 3:
                        src = wtile[0:1, 0, 0:2]
                    else:
                        src = wtile[0:1, 0:2].bitcast(BF16)[:, 1::2]
                    nc.tensor.matmul(scr, src, src, start=(i == 0), stop=(i == 3))

                def emit_stats(ic):
                    """DMA + DVE stats chain for chunk ic -> (tok4, ms)."""
                    tok4 = abp.tile([128, 4, 512], F32, tag="tok4",
                                    name=f"tok4_{ic}")
                    nc.gpsimd.dma_start(out=tok4, in_=tok_r[ic])
                    stats = stp.tile([128, 4, 6], F32, tag="stats")
                    mv = stp.tile([128, 4, 2], F32, tag="mv")
                    ms = stp.tile([128, 4], F32, tag="ms")
                    for t in range(4):
                        nc.vector.bn_stats(stats[:, t, :], tok4[:, t, :])
                    for t in range(4):
                        nc.vector.bn_aggr(mv[:, t, :], stats[:, t, :])
                    # E[x^2] = mean^2 + var + eps
                    nc.vector.tensor_tensor(
                        out=ms, in0=mv[:, :, 0], in1=mv[:, :, 0], op=ALU.mult)
                    nc.vector.tensor_tensor(out=ms, in0=ms, in1=mv[:, :, 1],
                                            op=ALU.add)
                    return tok4, ms

                def emit_rsqrt(ms):
                    """rstd = 1/sqrt(ms), entirely on the DVE: quake bit
                    trick evaluated in float domain + two Newton steps.
                    Keeps the RMSNorm chain off the busy ACT queue."""
                    msf = stp.tile([128, 4], F32, tag="msf")
                    nc.vector.tensor_copy(msf, ms.bitcast(I32))  # int bits as f32
                    y0 = stp.tile([128, 4], F32, tag="y0")
                    nc.vector.tensor_scalar(
                        out=y0.bitcast(I32), in0=msf, scalar1=-0.5,
                        scalar2=1597463007.0, op0=ALU.mult, op1=ALU.add)
                    y = y0
                    for it_n in range(2):
                        ysq = stp.tile([128, 4], F32, tag=f"ysq{it_n}")
                        nc.vector.tensor_tensor(out=ysq, in0=y, in1=y, op=ALU.mult)
                        half = stp.tile([128, 4], F32, tag=f"half{it_n}")
                        nc.vector.scalar_tensor_tensor(
                            half, ysq, -0.5, ms, op0=ALU.mult, op1=ALU.mult)
                        y2 = stp.tile([128, 4], F32, tag=f"yn{it_n}")
                        nc.vector.scalar_tensor_tensor(
                            y2, half, 1.5, y, op0=ALU.add, op1=ALU.mult)
                        y = y2
                    return y

                for ic in range(NIC):
                    tok4, ms = emit_stats(ic)
                    if RSQRT_ON_DVE:
                        rstd = emit_rsqrt(ms)
                    else:
                        s_t = stp.tile([128, 4], F32, tag="s_t")
                        nc.scalar.activation(s_t, ms, AF.Sqrt, bias=eps_t,
                                             scale=1.0)
                        rstd = stp.tile([128, 4], F32, tag="rstd",
                                        name=f"rstd{ic}")
                        nc.vector.reciprocal(rstd, s_t)
                    xt = abp.tile([128, 4, 512], BF16, tag="xt")
                    xns = []
                    for t in range(4):
                        xn = stp.tile([128, 512], BF16, tag=f"xn{t}")
                        eng = nc.gpsimd if t != 3 else nc.vector
                        eng.tensor_scalar_mul(xn, in0=tok4[:, t, :],
                                              scalar1=rstd[:, t:t + 1])
                        xns.append(xn)
                    for t in range(4):
                        tp = abps.tile([128, 4, 128], BF16, tag="tp")
                        for c in range(4):
                            nc.tensor.transpose(tp[:, c, :], xns[t][:, c * 128:(c + 1) * 128], ident_b)
                        if t in XT_DVE_T:
                            nc.vector.tensor_copy(xt[:, :, t * 128:(t + 1) * 128], tp)
                        else:
                            nc.scalar.copy(xt[:, :, t * 128:(t + 1) * 128], tp)

                    # QKV^T for this 512-token chunk (V first, then Q, K)
                    vt = abp.tile([128, 512], F32R, tag="vt")
                    for wtile, dst in ((wv_sb, None), (wq_sb, QT), (wk_sb, KT)):
                        ps = qkps.tile([128, 512], F32, tag="qk")
                        for c in range(4):
                            nc.tensor.matmul(ps, wtile[:, c, :], xt[:, c, :],
                                             start=(c == 0), stop=(c == 3))
                        if dst is None:
                            nc.scalar.copy(vt, ps)
                        elif dst is QT and QT_ON_ACT:
                            nc.scalar.copy(dst[:, ic * 512:(ic + 1) * 512], ps)
                        else:
                            nc.scalar.copy(dst[:, ic * 512:(ic + 1) * 512], ps)
                    # V^T -> V natural (j on partitions) + bf16 convert into Vb
                    vtp = vps_pool.tile([128, 4, 128], F32R, tag="vp")
                    for jl in range(4):
                        nc.tensor.transpose(vtp[:, jl, :], vt[:, jl * 128:(jl + 1) * 128], ident)
                    nc.scalar.copy(
                        Vb[:, ic * 4:(ic + 1) * 4, :, 0:64],
                        vtp.rearrange("p jl (h v) -> p jl h v", h=2))

            # ---------------- phase B + C ----------------
            NG = 16  # 2-jt exp/S^T groups per (ib, hl)
            with tc.tile_pool(name="st_psum", bufs=3, space="PSUM") as stps, \
                 tc.tile_pool(name="o_psum", bufs=1, space="PSUM") as ops, \
                 tc.tile_pool(name="tail_psum", bufs=1, space="PSUM") as tailps, \
                 tc.tile_pool(name="c_sbuf", bufs=3) as cp, \
                 tc.tile_pool(name="osb_pool", bufs=2) as osbp, \
                 tc.tile_pool(name="lc_pool", bufs=4) as lcp, \
                 tc.tile_pool(name="outsb_pool", bufs=2) as outsbp:

                iters = [(ib, hl) for ib in range(NIB) for hl in range(2)]
                gcount = [0]
                def emit_st(ib, hl, g):
                    """S^T for jt = 2g, 2g+1 of query block ib, head hl."""
                    h0 = hl * 64
                    st = stps.tile([128, 2, 512], F32, tag="st", name="stg")
                    for k in range(2):
                        jt = 2 * g + k
                        nc.tensor.matmul(
                            st[:, k, :],
                            KT[h0:h0 + 64, jt * 128:(jt + 1) * 128],
                            QT[h0:h0 + 64, ib * 512:(ib + 1) * 512],
                            start=True, stop=True)
                    return st

                def emit_exp(st):
                    """exp of a 2-jt S^T group -> new pt tile (bf16)."""
                    pt = ptp.tile([128, 2, 512], BF16, tag="pt", name="ptg")
                    use_dve = (gcount[0] * DVE_EXP_OF256) % 256 < DVE_EXP_OF256
                    gcount[0] += 1
                    src = st.rearrange("p a b -> p (a b)")
                    if use_dve:
                        nc.vector.tensor_scalar(
                            out=pt.rearrange("p a b -> p (a b)").bitcast(I16),
                            in0=src, scalar1=EXPA, scalar2=EXPB,
                            op0=ALU.mult, op1=ALU.add)
                    else:
                        nc.scalar.activation(
                            pt.rearrange("p a b -> p (a b)"), src, AF.Exp)
                    return pt

                def emit_pv(o4, hl, g, pt):
                    for k in range(2):
                        jt = 2 * g + k
                        for it in range(4):
                            nc.tensor.matmul(
                                o4[:, it * 65:(it + 1) * 65],
                                pt[:, k, it * 128:(it + 1) * 128],
                                Vb[:, jt, hl, :],
                                start=(g == 0 and k == 0 and it == 0),
                                stop=(g == NG - 1 and k == 1 and it == 3),
                                skip_group_check=not (g == 0 and k == 0 and it == 0))
                    return o4

                saved = {}

                def emit_tail_hl(o4, ib, hl):
                    """linv + scaled copies o -> O_sb half for this head."""
                    if hl == 0:
                        osb = osbp.tile([128, 4, 128], F32R, tag="osb",
                                        name=f"osb{ib}")
                        saved[ib] = osb
                    else:
                        osb = saved[ib]
                    o4v = o4[:, 0:260].rearrange("p (t c) -> p t c", t=4)
                    linv = lcp.tile([128, 4, 1], F32, tag=f"lc{hl}")
                    nc.vector.reciprocal(linv, o4v[:, :, 64:65])
                    nc.vector.tensor_tensor(
                        out=osb[:, :, hl * 64:(hl + 1) * 64],
                        in0=o4v[:, :, 0:64],
                        in1=linv.broadcast_to([128, 4, 64]),
                        op=ALU.mult)

                def emit_ot(ib):
                    """transpose O_sb -> O^T (PSUM) -> OT_sb (SBUF)."""
                    osb = saved.pop(ib)
                    otp = tailps.tile([128, 4, 128], F32R, tag="tail", name=f"otp{ib}")
                    for it in range(4):
                        nc.tensor.matmul(otp[:, it, :], osb[:, it, :], ident,
                                         is_transpose=True,
                                         start=(it == 0), stop=(it == 3),
                                         skip_group_check=(it != 0))
                    ot_sb = cp.tile([128, 4, 128], F32R, tag="ot", name=f"ot{ib}")
                    nc.vector.tensor_copy(ot_sb, otp)
                    return ot_sb

                def emit_outproj(ib, it, ot_sb, out_sb):
                    op_ps = tailps.tile([128, 512], F32, tag="tail", name=f"op{ib}_{it}")
                    nc.tensor.matmul(op_ps, ot_sb[:, it, :], wo_sb,
                                     start=True, stop=True)
                    if it % 2 < OUT_COPY_DVE_MOD:
                        nc.vector.tensor_copy(out_sb[:, it, :], op_ps)
                    else:
                        nc.scalar.copy(out_sb[:, it, :], op_ps)
                    nc.gpsimd.dma_start(out=out_r[ib][:, it, :],
                                        in_=out_sb[:, it, :])

                # software pipeline over the flat group stream: at step s the
                # PE emits S^T(s+1), ACT/DVE emit exp(s), and the PE emits
                # PV(s-2) — so PV's stationary (pt) is always two full groups
                # old; the exp->PV semaphore handoff is fully hidden.
                steps = [(idx, g) for idx in range(len(iters)) for g in range(NG)]
                pv_queue = []        # (pt, idx, g) owed PVs (depth 2)
                prev_tail = None     # (o4, ib, hl) owed a tail
                pending = None       # ib owed transposes/out-proj
                ot_sbs = None
                out_sbs = {}
                o4s = {}
                st_cur = emit_st(iters[0][0], iters[0][1], 0)
                for s, (idx, g) in enumerate(steps):
                    ib, hl = iters[idx]
                    st = st_cur
                    pt = emit_exp(st)
                    if s + 1 < len(steps):
                        nidx, ng = steps[s + 1]
                        st_cur = emit_st(iters[nidx][0], iters[nidx][1], ng)
                    # out-proj staging for the previous completed ib
                    if g == 2 and prev_tail is not None:
                        if iters[idx - 1][1] == 1:
                            pending = iters[idx - 1][0]
                            out_sbs[pending] = outsbp.tile(
                                [128, 4, 512], F32, tag="out_sb",
                                name=f"outsb{pending}")
                        prev_tail = None
                    elif g == 3 and pending is not None:
                        ot_sbs = emit_ot(pending)
                    elif g in (4, 6, 8, 10) and pending is not None:
                        it = (g - 4) // 2
                        emit_outproj(pending, it, ot_sbs, out_sbs[pending])
                        if g == 10:
                            out_sbs.pop(pending)
                            pending = None
                    if len(pv_queue) >= 2:
                        ppt, pidx, pg = pv_queue.pop(0)
                        pib, phl = iters[pidx]
                        if pg == 0:
                            o4s[pidx] = ops.tile([128, 512], F32, tag="o",
                                                 name=f"o{pidx}")
                        emit_pv(o4s[pidx], phl, pg, ppt)
                        if pg == NG - 1:
                            # tail right after the last PV: a full step of
                            # slack before the next iteration's first PV
                            # re-starts the o bank
                            emit_tail_hl(o4s[pidx], pib, phl)
                            prev_tail = (o4s.pop(pidx), pib, phl)
                    pv_queue.append((pt, idx, g))

                # drain: remaining PV groups, then final tails
                for ppt, pidx, pg in pv_queue:
                    pib, phl = iters[pidx]
                    if pg == 0:
                        o4s[pidx] = ops.tile([128, 512], F32, tag="o",
                                             name=f"o{pidx}")
                    emit_pv(o4s[pidx], phl, pg, ppt)
                    if pg == NG - 1:
                        emit_tail_hl(o4s[pidx], pib, phl)
                        o4s.pop(pidx)
                fin = pib
                ot_fin = emit_ot(fin)
                out_fin = outsbp.tile([128, 4, 512], F32, tag="out_sb")
                for it in range(4):
                    emit_outproj(fin, it, ot_fin, out_fin)
            outer_ctx.close()

    fix_waits_nc(nc, mybir)
    return nc


def fix_waits_nc(nc, mybir):
    """Post-pass over the scheduled program: (1) remove semaphore waits that
    are transitively implied by earlier waits (Tile emits per-proc-minimal,
    not transitively-minimal, waits), (2) split any instruction still
    carrying more than one wait by injecting single-wait NoOps in front of
    it — this walrus build rejects >1 sync wait per instruction.
    Mutates nc in place so CoreSim and hardware run identical sync."""
    nop_id = [0]

    def _is_ge(w):
        return w.sync_type == "semaphore" and w.wait_mode == "sem-ge-imm"

    for fn in nc.m.functions:
        for blk in fn.blocks:
            insts = list(blk.instructions)
            n = len(insts)

            producers = {}
            cum = {}
            nonmono = set()  # sems ever decremented: counter logic invalid
            for idx, inst in enumerate(insts):
                si = inst.sync_info
                for u in (si.on_update if si else []) or []:
                    if u.sync_type != "semaphore":
                        continue
                    sid = u.id
                    if u.update_mode != "sem-inc":
                        nonmono.add(sid)
                        continue
                    cum[sid] = cum.get(sid, 0) + int(u.update_value)
                    producers.setdefault(sid, []).append((cum[sid], idx))

            def producer_of(sid, val):
                for cv, idx in producers.get(sid, ()):
                    if cv >= val:
                        return idx
                return None

            prev_eng = [None] * n
            last = {}
            for idx, inst in enumerate(insts):
                e = inst.engine
                prev_eng[idx] = last.get(e)
                last[e] = idx

            def get_waits(inst):
                si = inst.sync_info
                return list(si.on_wait) if si and si.on_wait else []

            def is_ge(w):
                return _is_ge(w) and w.id not in nonmono

            know = [dict() for _ in range(n)]
            for _ in range(3):
                changed = False
                for idx, inst in enumerate(insts):
                    k = dict(know[prev_eng[idx]]) if prev_eng[idx] is not None else {}
                    for w in get_waits(inst):
                        if not is_ge(w):
                            continue
                        sid, val = w.id, int(w.wait_value)
                        if k.get(sid, -1) < val:
                            k[sid] = val
                        p = producer_of(sid, val)
                        if p is not None:
                            for s2, v2 in know[p].items():
                                if k.get(s2, -1) < v2:
                                    k[s2] = v2
                    if k != know[idx]:
                        know[idx] = k
                        changed = True
                if not changed:
                    break

            new_insts = []
            dirty = False
            for idx, inst in enumerate(insts):
                si = inst.sync_info
                waits = get_waits(inst)
                if si is not None and waits:
                    base = dict(know[prev_eng[idx]]) if prev_eng[idx] is not None else {}
                    kept = []
                    for w in waits:
                        if is_ge(w):
                            sid, val = w.id, int(w.wait_value)
                            if base.get(sid, -1) >= val:
                                continue
                            base[sid] = val
                            p = producer_of(sid, val)
                            if p is not None:
                                for s2, v2 in know[p].items():
                                    if base.get(s2, -1) < v2:
                                        base[s2] = v2
                        kept.append(w)
                    if len(kept) != len(waits) or len(kept) > 1:
                        dirty = True
                        for w in kept[:-1]:
                            nop_id[0] += 1
                            nop = mybir.InstNoOp(
                                name=f"I-waitfix-{nop_id[0]}", ins=[], outs=[])
                            nop.engine = inst.engine
                            nop.sync_info = mybir.SyncInfo(on_wait=[w], on_update=[])
                            nc.register_instruction(nop)
                            new_insts.append(nop)
                        inst.sync_info = mybir.SyncInfo(
                            on_wait=kept[-1:],
                            on_update=list(si.on_update or []))
                new_insts.append(inst)
            if dirty:
                blk.instructions = new_insts


def get_program():
    if "nc" not in _prog_cache:
        _prog_cache["nc"] = build_program()
    return _prog_cache["nc"]


def _prep_inputs(tokens, norm_weight, w_qkv, w_out):
    tokens = np.ascontiguousarray(np.asarray(tokens, dtype=np.float32))
    norm_weight = np.asarray(norm_weight, dtype=np.float32)
    w_qkv = np.asarray(w_qkv, dtype=np.float32)
    w_out = np.asarray(w_out, dtype=np.float32)

    wp = w_qkv * norm_weight[:, None]  # fold RMSNorm weight into qkv weights

    in_maps = []
    for c in range(NCORES):
        b = c // 4
        h0 = 2 * (c % 4)
        m = {}
        m["tok"] = tokens[b]
        import ml_dtypes
        for name, off in (("wq", 0), ("wk", DI), ("wv", 2 * DI)):
            w = wp[:, off + h0 * DH: off + (h0 + 2) * DH]       # [512, 128]
            w = np.ascontiguousarray(
                w.reshape(4, 128, 128).transpose(1, 0, 2))       # [128, 4, 128]
            m[name] = w.astype(ml_dtypes.bfloat16)
        m["wo"] = np.ascontiguousarray(w_out[h0 * DH:(h0 + 2) * DH, :])  # [128, 512]
        in_maps.append(m)
    return in_maps


def run(tokens, norm_weight, w_qkv, w_out, trace=False):
    from concourse.bass_utils import run_bass_kernel_spmd
    nc = get_program()
    in_maps = _prep_inputs(tokens, norm_weight, w_qkv, w_out)
    res = run_bass_kernel_spmd(nc, in_maps, core_ids=list(range(NCORES)), trace=trace)
    parts = [res.results[c]["out_part"] for c in range(NCORES)]
    out = np.empty((B, N, D), dtype=np.float32)
    for b in range(B):
        out[b] = parts[4 * b] + parts[4 * b + 1] + parts[4 * b + 2] + parts[4 * b + 3]
    return out, res


def kernel(tokens, norm_weight, w_qkv, w_out):
    out, _ = run(tokens, norm_weight, w_qkv, w_out, trace=False)
    return out


# revision 42
# speedup vs baseline: 1.0028x; 1.0028x over previous
"""TRN2 Bass kernel for nn_Attention (RMSNorm + QKV + softmax attention + out-proj).

Sharding: 8 cores = 2 batches x 4 head-pairs. Core c handles batch c//4 and
heads (2*(c%4), 2*(c%4)+1). Each core computes its partial out-projection
(contracting only its 128 rows of dim_inner); host sums the 4 partials per batch.

Per-core pipeline (cost-model-aware layout):
  A) stream tokens [128,512] blocks: RMSNorm stats on DVE, rstd scale, PE
     transpose -> x^T [512, 4096]; Q^T,K^T = W^T x^T (PE);
     V natural [tok, dv] via stationary-x^T matmuls, stored bf16 with a ones
     column per head (softmax denominator comes out on query partitions).
  B) flash attention per (ib=512 queries, head):
     S^T tiles [128 j, 512 i] on PE -> exp split between ACT (table exp) and
     DVE (Schraudolph int-trick exp: one fused mult+add with int32 convert,
     bits reinterpreted as f32) -> PV in the *transposed* layout:
     stationary = P^T tile [128 j, 128 i], moving = V [128 j, 65] bf16,
     out = o[128 i, 65] accumulated over j in a single PSUM bank group.
     Column 64 = softmax denominator l_i per query (on partitions!).
  C) tail per (ib, head): linv = 1/l (DVE, strided over the 4 it-slices),
     scaled copies o*linv -> O_sb [128 i, 128 d]; per ib: PE transpose
     O_sb -> O^T, out-proj out[i,:] = O^T.T @ Wo (contraction over both
     heads at once), copy to SBUF, DMA partial out.

Engine discipline: single-wait walrus constraint handled by _patch_drain +
fix_waits_nc (unchanged from previous version).
"""
import sys
sys.path.insert(0, "/opt/trn_rl_repo")
import numpy as np

B, N, D = 2, 4096, 512
H, DH = 8, 64
DI = H * DH
NCORES = 8
EPS = 1.1920929e-07  # float32 eps (torch nn.RMSNorm default)

# Schraudolph exp constants, bf16 flavor: exp(x) ~= bitcast_bf16(int16(
# x*EXPA + EXPB)). EXPA = 2^7/ln2; EXPB = 127*2^7 - C with C tuned for
# minimax relative error given the truncating f32->i16 conversion
# (~3.3% max rel err incl. bf16 mantissa quantization).
EXPA = 184.6649652337873
EXPB = float(127 * (1 << 7)) - 5.1

# exp work split: out of every 256 consecutive 2-jt groups, this many go to
# the DVE (Schraudolph); the rest go to ACT (exact table exp).
DVE_EXP_OF256 = 92
# engine assignment knobs (tuned against the cost-model timeline)
QT_ON_ACT = True      # QT copy on ACT (else DVE)
XT_DVE_T = (0, 1, 2, 3)  # which t-slices of the xt copy go to DVE (else ACT)
RSQRT_ON_DVE = False
OUT_COPY_DVE_MOD = 2  # out copies: it % 2 < this -> DVE, else ACT (2=all DVE)

_prog_cache = {}


def _patch_drain(tile_mod, mybir):
    """Split the multi-wait tail drain into a chain of single-wait drains
    (this walrus build rejects >1 sync wait per instruction)."""
    if getattr(tile_mod.TileContext, "_drain_patched", False):
        return

    def _patched(self, tick_clock, wait_clock):
        from concourse.vector_clock import ScopedClock
        nc = self.nc
        drain_inst = nc.sync.drain()
        wait_clock.add_sem_waits(drain_inst.ins, ScopedClock({None: tick_clock.global_clock}))
        si = drain_inst.ins.sync_info
        if si is not None and si.on_wait and len(si.on_wait) > 1:
            waits = list(si.on_wait)
            drain_inst.ins.sync_info = mybir.SyncInfo(
                on_wait=waits[:1], on_update=list(si.on_update or []))
            for w in waits[1:]:
                d2 = nc.sync.drain()
                d2.ins.sync_info = mybir.SyncInfo(on_wait=[w], on_update=[])
        nc.all_engine_barrier()
        assert self.sems is not None
        popped = nc._tile_sem_poison_stack.pop()
        assert popped is self._sem_poison
        nc.clear_and_free_semaphores(list(self.sems.allocated().values()))
        nc.all_engine_barrier()

    tile_mod.TileContext._drain_and_barrier = _patched
    tile_mod.TileContext._drain_patched = True


def build_program():
    import concourse.bass as bass
    import concourse.tile as tile
    from concourse import mybir
    from concourse.masks import make_identity

    _patch_drain(tile, mybir)

    F32 = mybir.dt.float32
    F32R = mybir.dt.float32r
    BF16 = mybir.dt.bfloat16
    I16 = mybir.dt.int16
    I32 = mybir.dt.int32
    AF = mybir.ActivationFunctionType
    ALU = mybir.AluOpType

    NIC = N // 512          # 8 chunks of 512 tokens
    NJT = N // 128          # 32 key tiles of 128
    NIB = N // 512          # 8 query blocks of 512

    nc = bass.Bass(trn_type="TRN2", target_bir_lowering=False)

    tok = nc.dram_tensor("tok", [N, D], F32, kind="ExternalInput")
    wq = nc.dram_tensor("wq", [128, 4, 128], BF16, kind="ExternalInput")
    wk = nc.dram_tensor("wk", [128, 4, 128], BF16, kind="ExternalInput")
    wv = nc.dram_tensor("wv", [128, 4, 128], BF16, kind="ExternalInput")
    wo = nc.dram_tensor("wo", [128, 512], F32R, kind="ExternalInput")
    out_part = nc.dram_tensor("out_part", [N, D], F32, kind="ExternalOutput")

    tok_r = tok.rearrange("(ic t p) d -> ic p t d", t=4, p=128)
    out_r = out_part.rearrange("(ib t p) e -> ib p t e", t=4, p=128)

    with tile.TileContext(nc) as tc:
        with tc.tile_pool(name="consts", bufs=1) as consts, \
             tc.tile_pool(name="big", bufs=1) as big, \
             tc.tile_pool(name="wpool", bufs=1) as wpool:

            # ---- constants ----
            ident_f = consts.tile([128, 128], F32)
            make_identity(nc, ident_f)
            ident = consts.tile([128, 128], F32R)
            nc.vector.tensor_copy(ident, ident_f)
            ident_b = consts.tile([128, 128], BF16)
            nc.vector.tensor_copy(ident_b, ident_f)
            eps_t = consts.tile([128, 1], F32)
            nc.vector.memset(eps_t, EPS)
            # preload the Exp ACT table during the otherwise-idle start window
            warm = consts.tile([1, 1], F32)
            nc.scalar.activation(warm, eps_t[0:1, :], AF.Exp)

            # ---- weights ----
            wq_sb = wpool.tile([128, 4, 128], BF16)
            wk_sb = wpool.tile([128, 4, 128], BF16)
            wv_sb = wpool.tile([128, 4, 128], BF16)
            wo_sb = wpool.tile([128, 512], F32R)
            nc.sync.dma_start(out=wq_sb, in_=wq[:, :, :])
            nc.sync.dma_start(out=wk_sb, in_=wk[:, :, :])
            nc.sync.dma_start(out=wv_sb, in_=wv[:, :, :])
            nc.sync.dma_start(out=wo_sb, in_=wo[:, :])

            # ---- persistent big buffers ----
            QT = big.tile([128, N], F32R)       # [2 heads x 64 qdims, n]
            KT = big.tile([128, N], F32R)
            Vb = big.tile([128, NJT, 2, 65], BF16)  # per j-tile: [v(64)|ones] per head
            nc.vector.memset(Vb[:, :, :, 64:65], 1.0)

            from contextlib import ExitStack
            outer_ctx = ExitStack()
            ptp = outer_ctx.enter_context(tc.tile_pool(name="pt_pool", bufs=5))

            # ---------------- phase A ----------------
            with tc.tile_pool(name="ab_sbuf", bufs=4) as abp, \
                 tc.tile_pool(name="ab_stats", bufs=6) as stp, \
                 tc.tile_pool(name="ab_psum", bufs=3, space="PSUM") as abps, \
                 tc.tile_pool(name="qk_psum", bufs=3, space="PSUM") as qkps, \
                 tc.tile_pool(name="v_psum", bufs=1, space="PSUM") as vps_pool, \
                 tc.tile_pool(name="scr_psum", bufs=1, space="PSUM") as scrps:

                # PE joins: absorb each weight-DMA semaphore with a tiny bf16 matmul
                scr = scrps.tile([2, 2], F32, tag="scr", name="scrj")
                for i, wtile in enumerate((wq_sb, wk_sb, wv_sb, wo_sb)):
                    if len(wtile.shape) == 3:
                        src = wtile[0:1, 0, 0:2]
                    else:
                        src = wtile[0:1, 0:2].bitcast(BF16)[:, 1::2]
                    nc.tensor.matmul(scr, src, src, start=(i == 0), stop=(i == 3))

                def emit_stats(ic):
                    """DMA + DVE stats chain for chunk ic -> (tok4, ms)."""
                    tok4 = abp.tile([128, 4, 512], F32, tag="tok4",
                                    name=f"tok4_{ic}")
                    if ic == 0:
                        # per-t DMAs: lets the stats chain start ~2us earlier
                        for t in range(4):
                            nc.sync.dma_start(out=tok4[:, t, :],
                                              in_=tok_r[ic][:, t, :])
                    else:
                        nc.sync.dma_start(out=tok4, in_=tok_r[ic])
                    stats = stp.tile([128, 4, 6], F32, tag="stats")
                    mv = stp.tile([128, 4, 2], F32, tag="mv")
                    ms = stp.tile([128, 4], F32, tag="ms")
                    for t in range(4):
                        nc.vector.bn_stats(stats[:, t, :], tok4[:, t, :])
                    for t in range(4):
                        nc.vector.bn_aggr(mv[:, t, :], stats[:, t, :])
                    # E[x^2] = mean^2 + var + eps
                    nc.vector.tensor_tensor(
                        out=ms, in0=mv[:, :, 0], in1=mv[:, :, 0], op=ALU.mult)
                    nc.vector.tensor_tensor(out=ms, in0=ms, in1=mv[:, :, 1],
                                            op=ALU.add)
                    return tok4, ms

                def emit_rsqrt(ms):
                    """rstd = 1/sqrt(ms), entirely on the DVE: quake bit
                    trick evaluated in float domain + two Newton steps.
                    Keeps the RMSNorm chain off the busy ACT queue."""
                    msf = stp.tile([128, 4], F32, tag="msf")
                    nc.vector.tensor_copy(msf, ms.bitcast(I32))  # int bits as f32
                    y0 = stp.tile([128, 4], F32, tag="y0")
                    nc.vector.tensor_scalar(
                        out=y0.bitcast(I32), in0=msf, scalar1=-0.5,
                        scalar2=1597463007.0, op0=ALU.mult, op1=ALU.add)
                    y = y0
                    for it_n in range(2):
                        ysq = stp.tile([128, 4], F32, tag=f"ysq{it_n}")
                        nc.vector.tensor_tensor(out=ysq, in0=y, in1=y, op=ALU.mult)
                        half = stp.tile([128, 4], F32, tag=f"half{it_n}")
                        nc.vector.scalar_tensor_tensor(
                            half, ysq, -0.5, ms, op0=ALU.mult, op1=ALU.mult)
                        y2 = stp.tile([128, 4], F32, tag=f"yn{it_n}")
                        nc.vector.scalar_tensor_tensor(
                            y2, half, 1.5, y, op0=ALU.add, op1=ALU.mult)
                        y = y2
                    return y

                for ic in range(NIC):
                    tok4, ms = emit_stats(ic)
                    if RSQRT_ON_DVE:
                        rstd = emit_rsqrt(ms)
                    else:
                        s_t = stp.tile([128, 4], F32, tag="s_t")
                        nc.scalar.activation(s_t, ms, AF.Sqrt, bias=eps_t,
                                             scale=1.0)
                        rstd = stp.tile([128, 4], F32, tag="rstd",
                                        name=f"rstd{ic}")
                        nc.vector.reciprocal(rstd, s_t)
                    xt = abp.tile([128, 4, 512], BF16, tag="xt")
                    xns = []
                    for t in range(4):
                        xn = stp.tile([128, 512], BF16, tag=f"xn{t}")
                        nc.gpsimd.tensor_scalar_mul(xn, in0=tok4[:, t, :],
                                                    scalar1=rstd[:, t:t + 1])
                        xns.append(xn)
                    for t in range(4):
                        tp = abps.tile([128, 4, 128], BF16, tag="tp")
                        for c in range(4):
                            nc.tensor.transpose(tp[:, c, :], xns[t][:, c * 128:(c + 1) * 128], ident_b)
                        if t in XT_DVE_T:
                            nc.vector.tensor_copy(xt[:, :, t * 128:(t + 1) * 128], tp)
                        else:
                            nc.scalar.copy(xt[:, :, t * 128:(t + 1) * 128], tp)

                    # QKV^T for this 512-token chunk (V first, then Q, K)
                    vt = abp.tile([128, 512], F32R, tag="vt")
                    for wtile, dst in ((wv_sb, None), (wq_sb, QT), (wk_sb, KT)):
                        ps = qkps.tile([128, 512], F32, tag="qk")
                        for c in range(4):
                            nc.tensor.matmul(ps, wtile[:, c, :], xt[:, c, :],
                                             start=(c == 0), stop=(c == 3))
                        if dst is None:
                            nc.scalar.copy(vt, ps)
                        elif dst is QT and QT_ON_ACT:
                            nc.scalar.copy(dst[:, ic * 512:(ic + 1) * 512], ps)
                        else:
                            nc.scalar.copy(dst[:, ic * 512:(ic + 1) * 512], ps)
                    # V^T -> V natural (j on partitions) + bf16 convert into Vb
                    vtp = vps_pool.tile([128, 4, 128], F32R, tag="vp")
                    for jl in range(4):
                        nc.tensor.transpose(vtp[:, jl, :], vt[:, jl * 128:(jl + 1) * 128], ident)
                    nc.scalar.copy(
                        Vb[:, ic * 4:(ic + 1) * 4, :, 0:64],
                        vtp.rearrange("p jl (h v) -> p jl h v", h=2))

            # ---------------- phase B + C ----------------
            NG = 16  # 2-jt exp/S^T groups per (ib, hl)
            GROUPS = [[2 * g, 2 * g + 1] for g in range(NG)]
            with tc.tile_pool(name="st_psum", bufs=3, space="PSUM") as stps, \
                 tc.tile_pool(name="o_psum", bufs=1, space="PSUM") as ops, \
                 tc.tile_pool(name="tail_psum", bufs=1, space="PSUM") as tailps, \
                 tc.tile_pool(name="c_sbuf", bufs=3) as cp, \
                 tc.tile_pool(name="osb_pool", bufs=2) as osbp, \
                 tc.tile_pool(name="lc_pool", bufs=4) as lcp, \
                 tc.tile_pool(name="outsb_pool", bufs=2) as outsbp:

                iters = [(ib, hl) for ib in range(NIB) for hl in range(2)]
                gcount = [0]
                def emit_st(ib, hl, g):
                    """S^T for the jt's of group g of query block ib."""
                    h0 = hl * 64
                    st = stps.tile([128, 2, 512], F32, tag="st", name="stg")
                    for k, jt in enumerate(GROUPS[g]):
                        nc.tensor.matmul(
                            st[:, k, :],
                            KT[h0:h0 + 64, jt * 128:(jt + 1) * 128],
                            QT[h0:h0 + 64, ib * 512:(ib + 1) * 512],
                            start=True, stop=True)
                    return st

                def emit_exp(st, g):
                    """exp of an S^T group -> new pt tile (bf16)."""
                    glen = len(GROUPS[g])
                    pt = ptp.tile([128, 2, 512], BF16, tag="pt", name="ptg")
                    use_dve = (gcount[0] * DVE_EXP_OF256) % 256 < DVE_EXP_OF256
                    gcount[0] += 1
                    src = st[:, 0:glen, :].rearrange("p a b -> p (a b)")
                    dst = pt[:, 0:glen, :].rearrange("p a b -> p (a b)")
                    if use_dve:
                        nc.vector.tensor_scalar(
                            out=dst.bitcast(I16),
                            in0=src, scalar1=EXPA, scalar2=EXPB,
                            op0=ALU.mult, op1=ALU.add)
                    else:
                        nc.scalar.activation(dst, src, AF.Exp)
                    return pt

                def emit_pv(o4, hl, g, pt):
                    glen = len(GROUPS[g])
                    for k, jt in enumerate(GROUPS[g]):
                        for it in range(4):
                            nc.tensor.matmul(
                                o4[:, it * 65:(it + 1) * 65],
                                pt[:, k, it * 128:(it + 1) * 128],
                                Vb[:, jt, hl, :],
                                start=(g == 0 and k == 0 and it == 0),
                                stop=(g == NG - 1 and k == glen - 1 and it == 3),
                                skip_group_check=not (g == 0 and k == 0 and it == 0))
                    return o4

                saved = {}

                def emit_tail_hl(o4, ib, hl):
                    """linv + scaled copies o -> O_sb half for this head."""
                    if hl == 0:
                        osb = osbp.tile([128, 4, 128], F32R, tag="osb",
                                        name=f"osb{ib}")
                        saved[ib] = osb
                    else:
                        osb = saved[ib]
                    o4v = o4[:, 0:260].rearrange("p (t c) -> p t c", t=4)
                    linv = lcp.tile([128, 4, 1], F32, tag=f"lc{hl}")
                    nc.vector.reciprocal(linv, o4v[:, :, 64:65])
                    nc.vector.tensor_tensor(
                        out=osb[:, :, hl * 64:(hl + 1) * 64],
                        in0=o4v[:, :, 0:64],
                        in1=linv.broadcast_to([128, 4, 64]),
                        op=ALU.mult)

                def emit_ot(ib):
                    """transpose O_sb -> O^T (PSUM) -> OT_sb (SBUF)."""
                    osb = saved.pop(ib)
                    otp = tailps.tile([128, 4, 128], F32R, tag="tail", name=f"otp{ib}")
                    for it in range(4):
                        nc.tensor.matmul(otp[:, it, :], osb[:, it, :], ident,
                                         is_transpose=True,
                                         start=(it == 0), stop=(it == 3),
                                         skip_group_check=(it != 0))
                    ot_sb = cp.tile([128, 4, 128], F32R, tag="ot", name=f"ot{ib}")
                    nc.vector.tensor_copy(ot_sb, otp)
                    return ot_sb

                def emit_outproj(ib, it, ot_sb, out_sb, alternate=False):
                    op_ps = tailps.tile([128, 512], F32, tag="tail", name=f"op{ib}_{it}")
                    nc.tensor.matmul(op_ps, ot_sb[:, it, :], wo_sb,
                                     start=True, stop=True)
                    on_dve = (it % 2 == 0) if alternate \
                        else (it % 2 < OUT_COPY_DVE_MOD)
                    if on_dve:
                        nc.vector.tensor_copy(out_sb[:, it, :], op_ps)
                    else:
                        nc.scalar.copy(out_sb[:, it, :], op_ps)
                    nc.sync.dma_start(out=out_r[ib][:, it, :],
                                      in_=out_sb[:, it, :])

                # software pipeline over the flat group stream: at step s the
                # PE emits S^T(s+1), ACT/DVE emit exp(s), and the PE emits
                # PV(s-2) — so PV's stationary (pt) is always two full groups
                # old; the exp->PV semaphore handoff is fully hidden.
                steps = [(idx, g) for idx in range(len(iters)) for g in range(NG)]
                pv_queue = []        # (pt, idx, g) owed PVs (depth 2)
                prev_tail = None     # (o4, ib, hl) owed a tail
                pending = None       # ib owed transposes/out-proj
                ot_sbs = None
                out_sbs = {}
                o4s = {}
                st_cur = emit_st(iters[0][0], iters[0][1], 0)
                for s, (idx, g) in enumerate(steps):
                    ib, hl = iters[idx]
                    st = st_cur
                    pt = emit_exp(st, g)
                    if s + 1 < len(steps):
                        nidx, ng = steps[s + 1]
                        st_cur = emit_st(iters[nidx][0], iters[nidx][1], ng)
                    # out-proj staging for the previous completed ib
                    if g == 2 and prev_tail is not None:
                        if iters[idx - 1][1] == 1:
                            pending = iters[idx - 1][0]
                            out_sbs[pending] = outsbp.tile(
                                [128, 4, 512], F32, tag="out_sb",
                                name=f"outsb{pending}")
                        prev_tail = None
                    elif g == 3 and pending is not None:
                        ot_sbs = emit_ot(pending)
                    elif g in (4, 6, 8, 10) and pending is not None:
                        it = (g - 4) // 2
                        emit_outproj(pending, it, ot_sbs, out_sbs[pending])
                        if g == 10:
                            out_sbs.pop(pending)
                            pending = None
                    if len(pv_queue) >= 2:
                        ppt, pidx, pg = pv_queue.pop(0)
                        pib, phl = iters[pidx]
                        if pg == 0:
                            o4s[pidx] = ops.tile([128, 512], F32, tag="o",
                                                 name=f"o{pidx}")
                        emit_pv(o4s[pidx], phl, pg, ppt)
                        if pg == NG - 1:
                            # tail right after the last PV: a full step of
                            # slack before the next iteration's first PV
                            # re-starts the o bank
                            emit_tail_hl(o4s[pidx], pib, phl)
                            prev_tail = (o4s.pop(pidx), pib, phl)
                    pv_queue.append((pt, idx, g))

                # drain: remaining PV groups, then final tails
                for ppt, pidx, pg in pv_queue:
                    pib, phl = iters[pidx]
                    if pg == 0:
                        o4s[pidx] = ops.tile([128, 512], F32, tag="o",
                                             name=f"o{pidx}")
                    emit_pv(o4s[pidx], phl, pg, ppt)
                    if pg == NG - 1:
                        emit_tail_hl(o4s[pidx], pib, phl)
                        o4s.pop(pidx)
                fin = pib
                ot_fin = emit_ot(fin)
                out_fin = outsbp.tile([128, 4, 512], F32, tag="out_sb")
                for it in range(4):
                    emit_outproj(fin, it, ot_fin, out_fin, alternate=True)
            outer_ctx.close()

    fix_waits_nc(nc, mybir)
    return nc


def fix_waits_nc(nc, mybir):
    """Post-pass over the scheduled program: (1) remove semaphore waits that
    are transitively implied by earlier waits (Tile emits per-proc-minimal,
    not transitively-minimal, waits), (2) split any instruction still
    carrying more than one wait by injecting single-wait NoOps in front of
    it — this walrus build rejects >1 sync wait per instruction.
    Mutates nc in place so CoreSim and hardware run identical sync."""
    nop_id = [0]

    def _is_ge(w):
        return w.sync_type == "semaphore" and w.wait_mode == "sem-ge-imm"

    for fn in nc.m.functions:
        for blk in fn.blocks:
            insts = list(blk.instructions)
            n = len(insts)

            producers = {}
            cum = {}
            nonmono = set()  # sems ever decremented: counter logic invalid
            for idx, inst in enumerate(insts):
                si = inst.sync_info
                for u in (si.on_update if si else []) or []:
                    if u.sync_type != "semaphore":
                        continue
                    sid = u.id
                    if u.update_mode != "sem-inc":
                        nonmono.add(sid)
                        continue
                    cum[sid] = cum.get(sid, 0) + int(u.update_value)
                    producers.setdefault(sid, []).append((cum[sid], idx))

            def producer_of(sid, val):
                for cv, idx in producers.get(sid, ()):
                    if cv >= val:
                        return idx
                return None

            prev_eng = [None] * n
            last = {}
            for idx, inst in enumerate(insts):
                e = inst.engine
                prev_eng[idx] = last.get(e)
                last[e] = idx

            def get_waits(inst):
                si = inst.sync_info
                return list(si.on_wait) if si and si.on_wait else []

            def is_ge(w):
                return _is_ge(w) and w.id not in nonmono

            know = [dict() for _ in range(n)]
            for _ in range(3):
                changed = False
                for idx, inst in enumerate(insts):
                    k = dict(know[prev_eng[idx]]) if prev_eng[idx] is not None else {}
                    for w in get_waits(inst):
                        if not is_ge(w):
                            continue
                        sid, val = w.id, int(w.wait_value)
                        if k.get(sid, -1) < val:
                            k[sid] = val
                        p = producer_of(sid, val)
                        if p is not None:
                            for s2, v2 in know[p].items():
                                if k.get(s2, -1) < v2:
                                    k[s2] = v2
                    if k != know[idx]:
                        know[idx] = k
                        changed = True
                if not changed:
                    break

            new_insts = []
            dirty = False
            for idx, inst in enumerate(insts):
                si = inst.sync_info
                waits = get_waits(inst)
                if si is not None and waits:
                    base = dict(know[prev_eng[idx]]) if prev_eng[idx] is not None else {}
                    kept = []
                    for w in waits:
                        if is_ge(w):
                            sid, val = w.id, int(w.wait_value)
                            if base.get(sid, -1) >= val:
                                continue
                            base[sid] = val
                            p = producer_of(sid, val)
                            if p is not None:
                                for s2, v2 in know[p].items():
                                    if base.get(s2, -1) < v2:
                                        base[s2] = v2
                        kept.append(w)
                    if len(kept) != len(waits) or len(kept) > 1:
                        dirty = True
                        for w in kept[:-1]:
                            nop_id[0] += 1
                            nop = mybir.InstNoOp(
                                name=f"I-waitfix-{nop_id[0]}", ins=[], outs=[])
                            nop.engine = inst.engine
                            nop.sync_info = mybir.SyncInfo(on_wait=[w], on_update=[])
                            nc.register_instruction(nop)
                            new_insts.append(nop)
                        inst.sync_info = mybir.SyncInfo(
                            on_wait=kept[-1:],
                            on_update=list(si.on_update or []))
                new_insts.append(inst)
            if dirty:
                blk.instructions = new_insts


def get_program():
    if "nc" not in _prog_cache:
        _prog_cache["nc"] = build_program()
    return _prog_cache["nc"]


def _prep_inputs(tokens, norm_weight, w_qkv, w_out):
    tokens = np.ascontiguousarray(np.asarray(tokens, dtype=np.float32))
    norm_weight = np.asarray(norm_weight, dtype=np.float32)
    w_qkv = np.asarray(w_qkv, dtype=np.float32)
    w_out = np.asarray(w_out, dtype=np.float32)

    wp = w_qkv * norm_weight[:, None]  # fold RMSNorm weight into qkv weights

    in_maps = []
    for c in range(NCORES):
        b = c // 4
        h0 = 2 * (c % 4)
        m = {}
        m["tok"] = tokens[b]
        import ml_dtypes
        for name, off in (("wq", 0), ("wk", DI), ("wv", 2 * DI)):
            w = wp[:, off + h0 * DH: off + (h0 + 2) * DH]       # [512, 128]
            w = np.ascontiguousarray(
                w.reshape(4, 128, 128).transpose(1, 0, 2))       # [128, 4, 128]
            m[name] = w.astype(ml_dtypes.bfloat16)
        m["wo"] = np.ascontiguousarray(w_out[h0 * DH:(h0 + 2) * DH, :])  # [128, 512]
        in_maps.append(m)
    return in_maps


def run(tokens, norm_weight, w_qkv, w_out, trace=False):
    from concourse.bass_utils import run_bass_kernel_spmd
    nc = get_program()
    in_maps = _prep_inputs(tokens, norm_weight, w_qkv, w_out)
    res = run_bass_kernel_spmd(nc, in_maps, core_ids=list(range(NCORES)), trace=trace)
    parts = [res.results[c]["out_part"] for c in range(NCORES)]
    out = np.empty((B, N, D), dtype=np.float32)
    for b in range(B):
        out[b] = parts[4 * b] + parts[4 * b + 1] + parts[4 * b + 2] + parts[4 * b + 3]
    return out, res


def kernel(tokens, norm_weight, w_qkv, w_out):
    out, _ = run(tokens, norm_weight, w_qkv, w_out, trace=False)
    return out


# revision 43
# speedup vs baseline: 1.0063x; 1.0035x over previous
"""TRN2 Bass kernel for nn_Attention (RMSNorm + QKV + softmax attention + out-proj).

Sharding: 8 cores = 2 batches x 4 head-pairs. Core c handles batch c//4 and
heads (2*(c%4), 2*(c%4)+1); each core computes its partial out-projection
(contracting its 128 rows of dim_inner); the host sums 4 partials per batch.

Per-core pipeline (engineered against the TimelineSim cost model):
  A) per 512-token chunk: RMSNorm stats on DVE (bn_stats), sqrt on ACT,
     rstd scale on GpSimd (bf16 out), PE transposes -> x^T (bf16),
     Q^T/K^T/V^T = W^T x^T (bf16 weights), V^T re-transposed to V-natural
     with a ones column per head, stored bf16 in Vb.
  B) flash attention, flat pipeline over 256 (ib, head, 2-jt) groups:
     PE computes S^T [128 j, 512 i] tiles (f32r, QT/KT kept f32r so scores
     stay full precision); exp is split ~62/38 between ACT (table exp,
     bf16 out) and DVE (Schraudolph int16 trick: one fused mult+add with
     truncating f32->i16 convert whose bits are the bf16 of exp(x));
     PV runs in the transposed layout: stationary = P^T tile [128 j, 128 i]
     (bf16), moving = V [128 j, 65] (bf16), out = o [128 i, 65] accumulated
     over all j in one PSUM-bank group per (ib, head) - output free size 65
     instead of 512 halves the PE cost, and the softmax denominator l lands
     on query partitions (column 64).
     Software pipeline: S^T runs one group ahead of exp; PV lags exp by two
     groups so the exp->PV semaphore handoff never stalls the PE.
  C) tail per (ib, head): linv = 1/l via one strided reciprocal, one
     broadcast tensor_tensor applies it while evacuating o -> O_sb; per ib:
     PE transposes O_sb -> O^T, out-proj out = O^T.T @ Wo (both heads in one
     contraction), PSUM->SBUF copy, DMA per i-tile (HWDGE/SP).

Engine budget per core (cost model): PE ~205us, ACT ~200us, DVE ~175us,
Pool ~45us, DMA ~50us -> 251.6us total. Error budget: bf16 x/W (~1.0%),
Schraudolph exp on 38% of scores (~0.4%), bf16 P/V (~0.3%) -> rel err
~1.2e-2 on the harness inputs (gate 2e-2).

The walrus build here rejects >1 semaphore wait per instruction
(_patch_drain + fix_waits_nc) and mixing 32-bit with non-32-bit matmul
inputs (hence all-bf16 or all-f32r operand pairs throughout).
"""
import sys
sys.path.insert(0, "/opt/trn_rl_repo")
import numpy as np

B, N, D = 2, 4096, 512
H, DH = 8, 64
DI = H * DH
NCORES = 8
EPS = 1.1920929e-07  # float32 eps (torch nn.RMSNorm default)

# Schraudolph exp constants, bf16 flavor: exp(x) ~= bitcast_bf16(int16(
# x*EXPA + EXPB)). EXPA = 2^7/ln2; EXPB = 127*2^7 - C with C tuned for
# minimax relative error given the truncating f32->i16 conversion
# (~3.3% max rel err incl. bf16 mantissa quantization).
EXPA = 184.6649652337873
EXPB = float(127 * (1 << 7)) - 5.1

# exp work split: out of every 256 consecutive 2-jt groups, this many go to
# the DVE (Schraudolph); the rest go to ACT (exact table exp).
DVE_EXP_OF256 = 96
# engine assignment knobs (tuned against the cost-model timeline)
QT_ON_ACT = True      # QT copy on ACT (else DVE)
XT_DVE_T = (0, 1, 2, 3)  # which t-slices of the xt copy go to DVE (else ACT)
RSQRT_ON_DVE = False
OUT_COPY_DVE_MOD = 2  # out copies: it % 2 < this -> DVE, else ACT (2=all DVE)

_prog_cache = {}


def _patch_drain(tile_mod, mybir):
    """Split the multi-wait tail drain into a chain of single-wait drains
    (this walrus build rejects >1 sync wait per instruction)."""
    if getattr(tile_mod.TileContext, "_drain_patched", False):
        return

    def _patched(self, tick_clock, wait_clock):
        from concourse.vector_clock import ScopedClock
        nc = self.nc
        drain_inst = nc.sync.drain()
        wait_clock.add_sem_waits(drain_inst.ins, ScopedClock({None: tick_clock.global_clock}))
        si = drain_inst.ins.sync_info
        if si is not None and si.on_wait and len(si.on_wait) > 1:
            waits = list(si.on_wait)
            drain_inst.ins.sync_info = mybir.SyncInfo(
                on_wait=waits[:1], on_update=list(si.on_update or []))
            for w in waits[1:]:
                d2 = nc.sync.drain()
                d2.ins.sync_info = mybir.SyncInfo(on_wait=[w], on_update=[])
        nc.all_engine_barrier()
        assert self.sems is not None
        popped = nc._tile_sem_poison_stack.pop()
        assert popped is self._sem_poison
        nc.clear_and_free_semaphores(list(self.sems.allocated().values()))
        nc.all_engine_barrier()

    tile_mod.TileContext._drain_and_barrier = _patched
    tile_mod.TileContext._drain_patched = True


def build_program():
    import concourse.bass as bass
    import concourse.tile as tile
    from concourse import mybir
    from concourse.masks import make_identity

    _patch_drain(tile, mybir)

    F32 = mybir.dt.float32
    F32R = mybir.dt.float32r
    BF16 = mybir.dt.bfloat16
    I16 = mybir.dt.int16
    I32 = mybir.dt.int32
    AF = mybir.ActivationFunctionType
    ALU = mybir.AluOpType

    NIC = N // 512          # 8 chunks of 512 tokens
    NJT = N // 128          # 32 key tiles of 128
    NIB = N // 512          # 8 query blocks of 512

    nc = bass.Bass(trn_type="TRN2", target_bir_lowering=False)

    tok = nc.dram_tensor("tok", [N, D], F32, kind="ExternalInput")
    wq = nc.dram_tensor("wq", [128, 4, 128], BF16, kind="ExternalInput")
    wk = nc.dram_tensor("wk", [128, 4, 128], BF16, kind="ExternalInput")
    wv = nc.dram_tensor("wv", [128, 4, 128], BF16, kind="ExternalInput")
    wo = nc.dram_tensor("wo", [128, 512], F32R, kind="ExternalInput")
    out_part = nc.dram_tensor("out_part", [N, D], F32, kind="ExternalOutput")

    tok_r = tok.rearrange("(ic t p) d -> ic p t d", t=4, p=128)
    out_r = out_part.rearrange("(ib t p) e -> ib p t e", t=4, p=128)

    with tile.TileContext(nc) as tc:
        with tc.tile_pool(name="consts", bufs=1) as consts, \
             tc.tile_pool(name="big", bufs=1) as big, \
             tc.tile_pool(name="wpool", bufs=1) as wpool:

            # ---- constants ----
            ident_f = consts.tile([128, 128], F32)
            make_identity(nc, ident_f)
            ident = consts.tile([128, 128], F32R)
            nc.vector.tensor_copy(ident, ident_f)
            ident_b = consts.tile([128, 128], BF16)
            nc.vector.tensor_copy(ident_b, ident_f)
            eps_t = consts.tile([128, 1], F32)
            nc.vector.memset(eps_t, EPS)
            # preload the Exp ACT table during the otherwise-idle start window
            warm = consts.tile([1, 1], F32)
            nc.scalar.activation(warm, eps_t[0:1, :], AF.Exp)

            # ---- weights ----
            wq_sb = wpool.tile([128, 4, 128], BF16)
            wk_sb = wpool.tile([128, 4, 128], BF16)
            wv_sb = wpool.tile([128, 4, 128], BF16)
            wo_sb = wpool.tile([128, 512], F32R)
            nc.sync.dma_start(out=wq_sb, in_=wq[:, :, :])
            nc.sync.dma_start(out=wk_sb, in_=wk[:, :, :])
            nc.sync.dma_start(out=wv_sb, in_=wv[:, :, :])
            nc.sync.dma_start(out=wo_sb, in_=wo[:, :])

            # ---- persistent big buffers ----
            QT = big.tile([128, N], F32R)       # [2 heads x 64 qdims, n]
            KT = big.tile([128, N], F32R)
            Vb = big.tile([128, NJT, 2, 65], BF16)  # per j-tile: [v(64)|ones] per head
            nc.vector.memset(Vb[:, :, :, 64:65], 1.0)

            from contextlib import ExitStack
            outer_ctx = ExitStack()
            ptp = outer_ctx.enter_context(tc.tile_pool(name="pt_pool", bufs=5))

            # ---------------- phase A ----------------
            with tc.tile_pool(name="ab_sbuf", bufs=4) as abp, \
                 tc.tile_pool(name="ab_stats", bufs=6) as stp, \
                 tc.tile_pool(name="ab_psum", bufs=3, space="PSUM") as abps, \
                 tc.tile_pool(name="qk_psum", bufs=3, space="PSUM") as qkps, \
                 tc.tile_pool(name="v_psum", bufs=1, space="PSUM") as vps_pool, \
                 tc.tile_pool(name="scr_psum", bufs=1, space="PSUM") as scrps:

                # PE joins: absorb each weight-DMA semaphore with a tiny bf16 matmul
                scr = scrps.tile([2, 2], F32, tag="scr", name="scrj")
                for i, wtile in enumerate((wq_sb, wk_sb, wv_sb, wo_sb)):
                    if len(wtile.shape) == 3:
                        src = wtile[0:1, 0, 0:2]
                    else:
                        src = wtile[0:1, 0:2].bitcast(BF16)[:, 1::2]
                    nc.tensor.matmul(scr, src, src, start=(i == 0), stop=(i == 3))

                def emit_stats(ic):
                    """DMA + DVE stats chain for chunk ic -> (tok4, ms)."""
                    tok4 = abp.tile([128, 4, 512], F32, tag="tok4",
                                    name=f"tok4_{ic}")
                    if ic == 0:
                        # per-t DMAs: lets the stats chain start ~2us earlier
                        for t in range(4):
                            nc.sync.dma_start(out=tok4[:, t, :],
                                              in_=tok_r[ic][:, t, :])
                    else:
                        nc.sync.dma_start(out=tok4, in_=tok_r[ic])
                    stats = stp.tile([128, 4, 6], F32, tag="stats")
                    mv = stp.tile([128, 4, 2], F32, tag="mv")
                    ms = stp.tile([128, 4], F32, tag="ms")
                    for t in range(4):
                        nc.vector.bn_stats(stats[:, t, :], tok4[:, t, :])
                    for t in range(4):
                        nc.vector.bn_aggr(mv[:, t, :], stats[:, t, :])
                    # E[x^2] = mean^2 + var + eps
                    nc.vector.tensor_tensor(
                        out=ms, in0=mv[:, :, 0], in1=mv[:, :, 0], op=ALU.mult)
                    nc.vector.tensor_tensor(out=ms, in0=ms, in1=mv[:, :, 1],
                                            op=ALU.add)
                    return tok4, ms

                def emit_rsqrt(ms):
                    """rstd = 1/sqrt(ms), entirely on the DVE: quake bit
                    trick evaluated in float domain + two Newton steps.
                    Keeps the RMSNorm chain off the busy ACT queue."""
                    msf = stp.tile([128, 4], F32, tag="msf")
                    nc.vector.tensor_copy(msf, ms.bitcast(I32))  # int bits as f32
                    y0 = stp.tile([128, 4], F32, tag="y0")
                    nc.vector.tensor_scalar(
                        out=y0.bitcast(I32), in0=msf, scalar1=-0.5,
                        scalar2=1597463007.0, op0=ALU.mult, op1=ALU.add)
                    y = y0
                    for it_n in range(2):
                        ysq = stp.tile([128, 4], F32, tag=f"ysq{it_n}")
                        nc.vector.tensor_tensor(out=ysq, in0=y, in1=y, op=ALU.mult)
                        half = stp.tile([128, 4], F32, tag=f"half{it_n}")
                        nc.vector.scalar_tensor_tensor(
                            half, ysq, -0.5, ms, op0=ALU.mult, op1=ALU.mult)
                        y2 = stp.tile([128, 4], F32, tag=f"yn{it_n}")
                        nc.vector.scalar_tensor_tensor(
                            y2, half, 1.5, y, op0=ALU.add, op1=ALU.mult)
                        y = y2
                    return y

                for ic in range(NIC):
                    tok4, ms = emit_stats(ic)
                    if RSQRT_ON_DVE:
                        rstd = emit_rsqrt(ms)
                    else:
                        s_t = stp.tile([128, 4], F32, tag="s_t")
                        nc.scalar.activation(s_t, ms, AF.Sqrt, bias=eps_t,
                                             scale=1.0)
                        rstd = stp.tile([128, 4], F32, tag="rstd",
                                        name=f"rstd{ic}")
                        nc.vector.reciprocal(rstd, s_t)
                    xt = abp.tile([128, 4, 512], BF16, tag="xt")
                    xns = []
                    for t in range(4):
                        xn = stp.tile([128, 512], BF16, tag=f"xn{t}")
                        nc.gpsimd.tensor_scalar_mul(xn, in0=tok4[:, t, :],
                                                    scalar1=rstd[:, t:t + 1])
                        xns.append(xn)
                    for t in range(4):
                        tp = abps.tile([128, 4, 128], BF16, tag="tp")
                        for c in range(4):
                            nc.tensor.transpose(tp[:, c, :], xns[t][:, c * 128:(c + 1) * 128], ident_b)
                        if t in XT_DVE_T:
                            nc.vector.tensor_copy(xt[:, :, t * 128:(t + 1) * 128], tp)
                        else:
                            nc.scalar.copy(xt[:, :, t * 128:(t + 1) * 128], tp)

                    # QKV^T for this 512-token chunk (V first, then Q, K)
                    vt = abp.tile([128, 512], F32R, tag="vt")
                    for wtile, dst in ((wv_sb, None), (wq_sb, QT), (wk_sb, KT)):
                        ps = qkps.tile([128, 512], F32, tag="qk")
                        for c in range(4):
                            nc.tensor.matmul(ps, wtile[:, c, :], xt[:, c, :],
                                             start=(c == 0), stop=(c == 3))
                        if dst is None:
                            nc.scalar.copy(vt, ps)
                        elif dst is QT and QT_ON_ACT:
                            nc.scalar.copy(dst[:, ic * 512:(ic + 1) * 512], ps)
                        else:
                            nc.scalar.copy(dst[:, ic * 512:(ic + 1) * 512], ps)
                    # V^T -> V natural (j on partitions) + bf16 convert into Vb
                    vtp = vps_pool.tile([128, 4, 128], F32R, tag="vp")
                    for jl in range(4):
                        nc.tensor.transpose(vtp[:, jl, :], vt[:, jl * 128:(jl + 1) * 128], ident)
                    nc.scalar.copy(
                        Vb[:, ic * 4:(ic + 1) * 4, :, 0:64],
                        vtp.rearrange("p jl (h v) -> p jl h v", h=2))

            # ---------------- phase B + C ----------------
            NG = 16  # 2-jt exp/S^T groups per (ib, hl)
            GROUPS = [[2 * g, 2 * g + 1] for g in range(NG)]
            with tc.tile_pool(name="st_psum", bufs=3, space="PSUM") as stps, \
                 tc.tile_pool(name="o_psum", bufs=1, space="PSUM") as ops, \
                 tc.tile_pool(name="tail_psum", bufs=1, space="PSUM") as tailps, \
                 tc.tile_pool(name="c_sbuf", bufs=3) as cp, \
                 tc.tile_pool(name="osb_pool", bufs=2) as osbp, \
                 tc.tile_pool(name="lc_pool", bufs=4) as lcp, \
                 tc.tile_pool(name="outsb_pool", bufs=2) as outsbp:

                iters = [(ib, hl) for ib in range(NIB) for hl in range(2)]
                gcount = [0]
                def emit_st(ib, hl, g):
                    """S^T for the jt's of group g of query block ib."""
                    h0 = hl * 64
                    st = stps.tile([128, 2, 512], F32, tag="st", name="stg")
                    for k, jt in enumerate(GROUPS[g]):
                        nc.tensor.matmul(
                            st[:, k, :],
                            KT[h0:h0 + 64, jt * 128:(jt + 1) * 128],
                            QT[h0:h0 + 64, ib * 512:(ib + 1) * 512],
                            start=True, stop=True)
                    return st

                def emit_exp(st, g):
                    """exp of an S^T group -> new pt tile (bf16)."""
                    glen = len(GROUPS[g])
                    pt = ptp.tile([128, 2, 512], BF16, tag="pt", name="ptg")
                    use_dve = (gcount[0] * DVE_EXP_OF256) % 256 < DVE_EXP_OF256
                    gcount[0] += 1
                    src = st[:, 0:glen, :].rearrange("p a b -> p (a b)")
                    dst = pt[:, 0:glen, :].rearrange("p a b -> p (a b)")
                    if use_dve:
                        nc.vector.tensor_scalar(
                            out=dst.bitcast(I16),
                            in0=src, scalar1=EXPA, scalar2=EXPB,
                            op0=ALU.mult, op1=ALU.add)
                    else:
                        nc.scalar.activation(dst, src, AF.Exp)
                    return pt

                def emit_pv(o4, hl, g, pt):
                    glen = len(GROUPS[g])
                    for k, jt in enumerate(GROUPS[g]):
                        for it in range(4):
                            nc.tensor.matmul(
                                o4[:, it * 65:(it + 1) * 65],
                                pt[:, k, it * 128:(it + 1) * 128],
                                Vb[:, jt, hl, :],
                                start=(g == 0 and k == 0 and it == 0),
                                stop=(g == NG - 1 and k == glen - 1 and it == 3),
                                skip_group_check=not (g == 0 and k == 0 and it == 0))
                    return o4

                saved = {}

                def emit_tail_hl(o4, ib, hl):
                    """linv + scaled copies o -> O_sb half for this head."""
                    if hl == 0:
                        osb = osbp.tile([128, 4, 128], F32R, tag="osb",
                                        name=f"osb{ib}")
                        saved[ib] = osb
                    else:
                        osb = saved[ib]
                    o4v = o4[:, 0:260].rearrange("p (t c) -> p t c", t=4)
                    linv = lcp.tile([128, 4, 1], F32, tag=f"lc{hl}")
                    nc.vector.reciprocal(linv, o4v[:, :, 64:65])
                    nc.vector.tensor_tensor(
                        out=osb[:, :, hl * 64:(hl + 1) * 64],
                        in0=o4v[:, :, 0:64],
                        in1=linv.broadcast_to([128, 4, 64]),
                        op=ALU.mult)

                def emit_ot(ib):
                    """transpose O_sb -> O^T (PSUM) -> OT_sb (SBUF)."""
                    osb = saved.pop(ib)
                    otp = tailps.tile([128, 4, 128], F32R, tag="tail", name=f"otp{ib}")
                    for it in range(4):
                        nc.tensor.matmul(otp[:, it, :], osb[:, it, :], ident,
                                         is_transpose=True,
                                         start=(it == 0), stop=(it == 3),
                                         skip_group_check=(it != 0))
                    ot_sb = cp.tile([128, 4, 128], F32R, tag="ot", name=f"ot{ib}")
                    nc.vector.tensor_copy(ot_sb, otp)
                    return ot_sb

                def emit_outproj(ib, it, ot_sb, out_sb, alternate=False):
                    op_ps = tailps.tile([128, 512], F32, tag="tail", name=f"op{ib}_{it}")
                    nc.tensor.matmul(op_ps, ot_sb[:, it, :], wo_sb,
                                     start=True, stop=True)
                    on_dve = (it % 2 == 0) if alternate \
                        else (it % 2 < OUT_COPY_DVE_MOD)
                    if on_dve:
                        nc.vector.tensor_copy(out_sb[:, it, :], op_ps)
                    else:
                        nc.scalar.copy(out_sb[:, it, :], op_ps)
                    nc.sync.dma_start(out=out_r[ib][:, it, :],
                                      in_=out_sb[:, it, :])

                # software pipeline over the flat group stream: at step s the
                # PE emits S^T(s+1), ACT/DVE emit exp(s), and the PE emits
                # PV(s-2) — so PV's stationary (pt) is always two full groups
                # old; the exp->PV semaphore handoff is fully hidden.
                steps = [(idx, g) for idx in range(len(iters)) for g in range(NG)]
                pv_queue = []        # (pt, idx, g) owed PVs (depth 2)
                prev_tail = None     # (o4, ib, hl) owed a tail
                pending = None       # ib owed transposes/out-proj
                ot_sbs = None
                out_sbs = {}
                o4s = {}
                st_cur = emit_st(iters[0][0], iters[0][1], 0)
                for s, (idx, g) in enumerate(steps):
                    ib, hl = iters[idx]
                    st = st_cur
                    pt = emit_exp(st, g)
                    if s + 1 < len(steps):
                        nidx, ng = steps[s + 1]
                        st_cur = emit_st(iters[nidx][0], iters[nidx][1], ng)
                    # out-proj staging for the previous completed ib
                    if g == 2 and prev_tail is not None:
                        if iters[idx - 1][1] == 1:
                            pending = iters[idx - 1][0]
                            out_sbs[pending] = outsbp.tile(
                                [128, 4, 512], F32, tag="out_sb",
                                name=f"outsb{pending}")
                        prev_tail = None
                    elif g == 3 and pending is not None:
                        ot_sbs = emit_ot(pending)
                    elif g in (4, 6, 8, 10) and pending is not None:
                        it = (g - 4) // 2
                        emit_outproj(pending, it, ot_sbs, out_sbs[pending])
                        if g == 10:
                            out_sbs.pop(pending)
                            pending = None
                    if len(pv_queue) >= 2:
                        ppt, pidx, pg = pv_queue.pop(0)
                        pib, phl = iters[pidx]
                        if pg == 0:
                            o4s[pidx] = ops.tile([128, 512], F32, tag="o",
                                                 name=f"o{pidx}")
                        emit_pv(o4s[pidx], phl, pg, ppt)
                        if pg == NG - 1:
                            # tail right after the last PV: a full step of
                            # slack before the next iteration's first PV
                            # re-starts the o bank
                            emit_tail_hl(o4s[pidx], pib, phl)
                            prev_tail = (o4s.pop(pidx), pib, phl)
                    pv_queue.append((pt, idx, g))

                # drain: remaining PV groups, then final tails
                for ppt, pidx, pg in pv_queue:
                    pib, phl = iters[pidx]
                    if pg == 0:
                        o4s[pidx] = ops.tile([128, 512], F32, tag="o",
                                             name=f"o{pidx}")
                    emit_pv(o4s[pidx], phl, pg, ppt)
                    if pg == NG - 1:
                        emit_tail_hl(o4s[pidx], pib, phl)
                        o4s.pop(pidx)
                fin = pib
                ot_fin = emit_ot(fin)
                out_fin = outsbp.tile([128, 4, 512], F32, tag="out_sb")
                for it in range(4):
                    emit_outproj(fin, it, ot_fin, out_fin, alternate=True)
            outer_ctx.close()

    fix_waits_nc(nc, mybir)
    return nc


def fix_waits_nc(nc, mybir):
    """Post-pass over the scheduled program: (1) remove semaphore waits that
    are transitively implied by earlier waits (Tile emits per-proc-minimal,
    not transitively-minimal, waits), (2) split any instruction still
    carrying more than one wait by injecting single-wait NoOps in front of
    it — this walrus build rejects >1 sync wait per instruction.
    Mutates nc in place so CoreSim and hardware run identical sync."""
    nop_id = [0]

    def _is_ge(w):
        return w.sync_type == "semaphore" and w.wait_mode == "sem-ge-imm"

    for fn in nc.m.functions:
        for blk in fn.blocks:
            insts = list(blk.instructions)
            n = len(insts)

            producers = {}
            cum = {}
            nonmono = set()  # sems ever decremented: counter logic invalid
            for idx, inst in enumerate(insts):
                si = inst.sync_info
                for u in (si.on_update if si else []) or []:
                    if u.sync_type != "semaphore":
                        continue
                    sid = u.id
                    if u.update_mode != "sem-inc":
                        nonmono.add(sid)
                        continue
                    cum[sid] = cum.get(sid, 0) + int(u.update_value)
                    producers.setdefault(sid, []).append((cum[sid], idx))

            def producer_of(sid, val):
                for cv, idx in producers.get(sid, ()):
                    if cv >= val:
                        return idx
                return None

            prev_eng = [None] * n
            last = {}
            for idx, inst in enumerate(insts):
                e = inst.engine
                prev_eng[idx] = last.get(e)
                last[e] = idx

            def get_waits(inst):
                si = inst.sync_info
                return list(si.on_wait) if si and si.on_wait else []

            def is_ge(w):
                return _is_ge(w) and w.id not in nonmono

            know = [dict() for _ in range(n)]
            for _ in range(3):
                changed = False
                for idx, inst in enumerate(insts):
                    k = dict(know[prev_eng[idx]]) if prev_eng[idx] is not None else {}
                    for w in get_waits(inst):
                        if not is_ge(w):
                            continue
                        sid, val = w.id, int(w.wait_value)
                        if k.get(sid, -1) < val:
                            k[sid] = val
                        p = producer_of(sid, val)
                        if p is not None:
                            for s2, v2 in know[p].items():
                                if k.get(s2, -1) < v2:
                                    k[s2] = v2
                    if k != know[idx]:
                        know[idx] = k
                        changed = True
                if not changed:
                    break

            new_insts = []
            dirty = False
            for idx, inst in enumerate(insts):
                si = inst.sync_info
                waits = get_waits(inst)
                if si is not None and waits:
                    base = dict(know[prev_eng[idx]]) if prev_eng[idx] is not None else {}
                    kept = []
                    for w in waits:
                        if is_ge(w):
                            sid, val = w.id, int(w.wait_value)
                            if base.get(sid, -1) >= val:
                                continue
                            base[sid] = val
                            p = producer_of(sid, val)
                            if p is not None:
                                for s2, v2 in know[p].items():
                                    if base.get(s2, -1) < v2:
                                        base[s2] = v2
                        kept.append(w)
                    if len(kept) != len(waits) or len(kept) > 1:
                        dirty = True
                        for w in kept[:-1]:
                            nop_id[0] += 1
                            nop = mybir.InstNoOp(
                                name=f"I-waitfix-{nop_id[0]}", ins=[], outs=[])
                            nop.engine = inst.engine
                            nop.sync_info = mybir.SyncInfo(on_wait=[w], on_update=[])
                            nc.register_instruction(nop)
                            new_insts.append(nop)
                        inst.sync_info = mybir.SyncInfo(
                            on_wait=kept[-1:],
                            on_update=list(si.on_update or []))
                new_insts.append(inst)
            if dirty:
                blk.instructions = new_insts


def get_program():
    if "nc" not in _prog_cache:
        _prog_cache["nc"] = build_program()
    return _prog_cache["nc"]


def _prep_inputs(tokens, norm_weight, w_qkv, w_out):
    tokens = np.ascontiguousarray(np.asarray(tokens, dtype=np.float32))
    norm_weight = np.asarray(norm_weight, dtype=np.float32)
    w_qkv = np.asarray(w_qkv, dtype=np.float32)
    w_out = np.asarray(w_out, dtype=np.float32)

    wp = w_qkv * norm_weight[:, None]  # fold RMSNorm weight into qkv weights

    in_maps = []
    for c in range(NCORES):
        b = c // 4
        h0 = 2 * (c % 4)
        m = {}
        m["tok"] = tokens[b]
        import ml_dtypes
        for name, off in (("wq", 0), ("wk", DI), ("wv", 2 * DI)):
            w = wp[:, off + h0 * DH: off + (h0 + 2) * DH]       # [512, 128]
            w = np.ascontiguousarray(
                w.reshape(4, 128, 128).transpose(1, 0, 2))       # [128, 4, 128]
            m[name] = w.astype(ml_dtypes.bfloat16)
        m["wo"] = np.ascontiguousarray(w_out[h0 * DH:(h0 + 2) * DH, :])  # [128, 512]
        in_maps.append(m)
    return in_maps


def run(tokens, norm_weight, w_qkv, w_out, trace=False):
    from concourse.bass_utils import run_bass_kernel_spmd
    nc = get_program()
    in_maps = _prep_inputs(tokens, norm_weight, w_qkv, w_out)
    res = run_bass_kernel_spmd(nc, in_maps, core_ids=list(range(NCORES)), trace=trace)
    parts = [res.results[c]["out_part"] for c in range(NCORES)]
    out = np.empty((B, N, D), dtype=np.float32)
    for b in range(B):
        out[b] = parts[4 * b] + parts[4 * b + 1] + parts[4 * b + 2] + parts[4 * b + 3]
    return out, res


def kernel(tokens, norm_weight, w_qkv, w_out):
    out, _ = run(tokens, norm_weight, w_qkv, w_out, trace=False)
    return out
